# revision 53
# baseline (speedup 1.0000x reference)
"""Trainium2 Bass kernel for an AttentionBlock (self-attn + cross-attn, pre-LN,
residuals), data-parallel over 8 NeuronCores.

Sharding: batch (4) x query-half (2) -> 8 cores. Each core computes 1024 query
rows end-to-end. Self-attention K/V are recomputed per core over the full 2048
rows of its batch (keys ordered [mine; other] -- softmax is permutation
invariant over keys). Cross-attention K/V come from the batch's 512 context
rows.

v3 layout strategy -- fp8 DoubleRow matmuls for every projection and for PV:
  - Host passes x / ctx / x^T / ctx^T in bf16 (LN stats from bf16 tiles), and
    all weights in fp8e4: wq/wk/wv prescaled by 256, wo by 32. The scale
    bookkeeping is folded into existing instructions (exp scale, exp bias for
    the fp8 prob prescale, the softmax-reciprocal scale, and a mult in the
    output sinks) -- zero extra ops.
  - LN: bn_stats on [token, feature] bf16 tiles; [128,2] stat columns are
    PE-transposed to rows and gpsimd-broadcast. z^T = (x^T - mean)*rstd in
    bf16 on DVE (2x); the per-feature affine runs on ScalarE as an Identity
    activation with per-partition scale/bias APs, writing fp8 pair tiles
    [128, 2, rows] so DoubleRow matmuls can contract 256 features per pass.
  - Projections (q/k/v, both attns): fp8 DoubleRow, psum carries 256-scaled
    outputs; k^T/q^T copied to bf16 (scale folded into the exp argument),
    v copied to fp8 head-major m-pair tiles [128, H, 2, 128] (dual-fp8
    ldweights needs a 128-multiple pair stride) with a 1/256 copy scale.
    LN gains are folded into the fp8 weights on the host (exact for any
    gamma); a separate has_bias build applies beta when any LN bias is
    nonzero.
  - Scores stay bf16: S^T = k^T.T @ q^T with zero-banded q (exact).
    exp on ScalarE: et = 16*exp(qk/8) in fp8 (16x keeps probs in e4m3's
    normal range; 240 clip is ~30x above the typical max exp).
  - PV: fp8 DoubleRow over m-pairs, psum rows 0..63 = 16*O, row 64 = 16*r
    (ones column in V). Normalize: DVE reciprocal_approx_fast (5x faster
    than reciprocal) + gpsimd broadcast + one scalar_tensor_tensor
    (pv * 8) * (1/(16r)) writing fp8 out tiles [128, 2, chunk].
  - Out-projections (both orientations) fp8 DoubleRow against 32-scaled wo;
    sinks divide the 256x psum scale inside the existing add ops.
"""

import sys

if '/opt/trn_rl_repo' not in sys.path:
    sys.path.insert(0, '/opt/trn_rl_repo')

import math

import numpy as np
import ml_dtypes

import concourse.bass as bass
import concourse.bacc as bacc
import concourse.tile as tile
import concourse.mybir as mybir
from concourse.masks import make_identity

F32 = mybir.dt.float32
BF16 = mybir.dt.bfloat16
FP8 = mybir.dt.float8e4
AX = mybir.AluOpType
AF = mybir.ActivationFunctionType
DR = mybir.MatmulPerfMode.DoubleRow

P = 128
D = 64          # head dim
EPS = 1e-5
SCALE = 0.125   # D ** -0.5

WS = 256.0      # wq/wk/wv host prescale
WOS = 32.0      # wo host prescale
PS = 16.0       # fp8 prob prescale (via exp bias)
OTS = 8.0       # fp8 attn-out prescale
ESCALE = SCALE / (WS * WS)          # exp scale: undo q,k 256x
EBIAS = math.log(PS)                # exp bias: prob prescale
SINKS = 1.0 / (OTS * WOS)           # sink scale: undo ot*wo prescale

DBG_REPS = 1    # repeat whole body inside one NEFF (timing)
DBG_SALT = 0    # pad blob32 length to defeat structure-keyed NEFF cache


class Cfg:
    def __init__(self, F=1024, CF=768, T=1024, MC=512, H=8):
        self.F = F                  # model features
        self.CF = CF                # context features
        self.T = T                  # my query rows
        self.M = 2 * T              # self-attn keys (mine + other)
        self.MC = MC                # ctx keys
        self.H = H                  # heads
        self.MID = H * D
        self.FB = F // P
        self.CFB = CF // P
        self.OB = self.MID // P     # qkv output blocks (2 heads each)
        self.TB = T // P
        self.MT = self.M // P
        self.CTB = MC // P
        self.TCHUNK = min(512, T)
        self.NTC = T // self.TCHUNK


def layout32(c):
    L, off = {}, 0
    for name, size in [
            ('sa_gq', P * c.FB), ('sa_bq', P * c.FB),
            ('sa_gkv', P * c.FB), ('sa_bkv', P * c.FB),
            ('ca_gq', P * c.FB), ('ca_bq', P * c.FB),
            ('ca_gkv', P * c.CFB), ('ca_bkv', P * c.CFB),
            ('sa_bo', c.F), ('ca_bo', c.F),
            ('sa_bo_col', P * c.FB), ('ca_bo_col', P * c.FB)]:
        L[name] = (off, size)
        off += size
    return L, off + DBG_SALT


def layout16(c):
    L, off = {}, 0
    for name, size in [
            ('x_mine', c.T * c.F), ('x_other', c.T * c.F),
            ('ctx', c.MC * c.CF),
            ('xT', c.F * c.M), ('ctxT', c.CF * c.MC),
            ('sa_bo16', c.F), ('ca_bo16', c.F)]:
        L[name] = (off, size)
        off += size
    return L, off


def layout8(c):
    L, off = {}, 0
    for name, size in [
            ('sa_wq', c.F * c.MID), ('sa_wk', c.F * c.MID),
            ('sa_wv', c.F * c.MID), ('sa_wo', c.MID * c.F),
            ('ca_wq', c.F * c.MID), ('ca_wk', c.CF * c.MID),
            ('ca_wv', c.CF * c.MID), ('ca_wo', c.MID * c.F)]:
        L[name] = (off, size)
        off += size
    return L, off


def _pbcast(nc, out, row):
    nc.gpsimd.partition_broadcast(out, row)


def _stats_cols(nc, sb_stats, xt, fdim, eps_t, dst_col):
    """LN stats of xt [128, fdim] -> dst_col [128, 33]: col 0 = mean,
    col 32 = rstd (32-aligned so the transposed rows are legal AP bases)."""
    g = (fdim + 511) // 512
    gd = fdim // g
    st6 = sb_stats.tile([P, g, 6], F32, tag="st6", name="st6")
    for gi in range(g):
        nc.vector.bn_stats(st6[:, gi:gi + 1, :],
                           xt[:, gi * gd:(gi + 1) * gd])
    st2 = sb_stats.tile([P, 2], F32, tag="st2", name="st2")
    nc.vector.bn_aggr(st2[:], st6[:])
    nc.vector.tensor_copy(dst_col[:, 0:1], st2[:, 0:1])
    sd = sb_stats.tile([P, 1], F32, tag="sd", name="sd")
    nc.scalar.activation(sd[:], st2[:, 1:2], AF.Sqrt, bias=eps_t[:])
    nc.vector.reciprocal(dst_col[:, 32:33], sd[:])


def build(nc, cfg, has_bias=False):
    c = cfg
    # ------- DRAM I/O (packed blobs to minimize tensor count) -------
    L32, N32 = layout32(c)
    L16, N16 = layout16(c)
    L8, N8 = layout8(c)
    blob32 = nc.dram_tensor("blob32", [N32], F32, kind="ExternalInput")
    blob16 = nc.dram_tensor("blob16", [N16], BF16, kind="ExternalInput")
    blob8 = nc.dram_tensor("blob8", [N8], FP8, kind="ExternalInput")
    out_d = nc.dram_tensor("out", [c.T, c.F], F32, kind="ExternalOutput")

    def g32(name):
        off, size = L32[name]
        return blob32.ap()[off:off + size]

    def g16(name):
        off, size = L16[name]
        return blob16.ap()[off:off + size]

    def g8(name):
        off, size = L8[name]
        return blob8.ap()[off:off + size]

    NCW = min(512, c.F)
    NC2 = c.F // NCW                 # n-chunks for out-proj
    TPC = c.TCHUNK // P              # row tiles per t-chunk

    with tile.TileContext(nc) as tc:
      for _rep in range(DBG_REPS):
        with tc.tile_pool(name="p_ln", bufs=1) as p_ln, \
             tc.tile_pool(name="p_kv", bufs=1) as p_kv:

            # ---- constants: LN params, bo broadcast, identity ----
            def ln_tile(name, fb):
                t = p_ln.tile([P, fb], F32, name=name + "_sb", tag=name)
                nc.sync.dma_start(t[:], g32(name).rearrange(
                    "(p a) -> p a", a=fb))
                return t

            sa_gq_t, sa_bq_t = ln_tile('sa_gq', c.FB), ln_tile('sa_bq', c.FB)
            sa_gkv_t, sa_bkv_t = (ln_tile('sa_gkv', c.FB),
                                  ln_tile('sa_bkv', c.FB))
            ca_gq_t, ca_bq_t = ln_tile('ca_gq', c.FB), ln_tile('ca_bq', c.FB)
            ca_gkv_t, ca_bkv_t = (ln_tile('ca_gkv', c.CFB),
                                  ln_tile('ca_bkv', c.CFB))
            sa_bo_col = ln_tile('sa_bo_col', c.FB)
            ca_bo_col = ln_tile('ca_bo_col', c.FB)

            eps_t = p_ln.tile([P, 1], F32, name="eps_t")
            nc.vector.memset(eps_t[:], EPS)
            ebias_t = p_ln.tile([P, 1], F32, name="ebias_t")
            nc.vector.memset(ebias_t[:], EBIAS)
            ident = p_ln.tile([P, P], F32, name="ident")
            make_identity(nc, ident[:])

            # self-attn K^T (bf16) / V (fp8 m-pairs) / q^T (bf16) storage
            kT = [p_kv.tile([P, c.M], BF16, tag="kT", bufs=c.OB,
                            name=f"kT{ob}") for ob in range(c.OB)]
            # v m-pair tiles, head-major [P, H, 2, 128]: dual-fp8 ldweights
            # needs a 128-multiple pair stride; cols 64.. stay 1.0 (denom)
            vv = [p_kv.tile([P, c.H, 2, P], FP8, tag="v",
                            bufs=c.MT // 2, name=f"v{m}")
                  for m in range(c.MT // 2)]
            qTz = [[p_kv.tile([P, c.T], BF16, tag="qTz", bufs=2 * c.OB,
                              name=f"qTz{par}_{ob}") for ob in range(c.OB)]
                   for par in range(2)]
            for ob in range(c.OB):
                nc.vector.memset(qTz[0][ob][D:P, :], 0.0)
                nc.vector.memset(qTz[1][ob][0:D, :], 0.0)
            for vt in vv:
                nc.gpsimd.memset(vt[:, :, :, D:D + 1], 1.0)

            def load_w_in(pool, name, fb):
                # [fb*P, MID] -> [P, fb*MID], fb-major
                t = pool.tile([P, fb * c.MID], FP8, name=name + "_sb",
                              tag=name)
                nc.sync.dma_start(
                    t[:].rearrange("p (a o) -> p a o", a=fb),
                    g8(name).rearrange("(a p o) -> p a o", p=P, o=c.MID))
                return t

            def load_w_out(pool, name):
                # [MID, F] -> [P, OB*F]
                t = pool.tile([P, c.OB * c.F], FP8, name=name + "_sb",
                              tag=name)
                nc.sync.dma_start(
                    t[:].rearrange("p (a f) -> p a f", a=c.OB),
                    g8(name).rearrange("(a p f) -> p a f", p=P, f=c.F))
                return t

            p_wl = tc.alloc_tile_pool(name="p_wl", bufs=1)
            # cross-attention storage + ctx weights first: ctx K/V is
            # independent of x, so the c0 phase can fill the startup window
            p_kvx = tc.alloc_tile_pool(name="p_kvx", bufs=1)
            ckT = [p_kvx.tile([P, c.MC], BF16, tag="ckT", bufs=c.OB,
                              name=f"ckT{ob}") for ob in range(c.OB)]
            cvv = [p_kvx.tile([P, c.H, 2, P], FP8, tag="cv",
                              bufs=c.CTB // 2, name=f"cv{m}")
                   for m in range(c.CTB // 2)]
            cqTz = [[p_kvx.tile([P, c.T], BF16, tag="cqTz", bufs=2 * c.OB,
                                name=f"cqTz{par}_{ob}")
                     for ob in range(c.OB)] for par in range(2)]
            for ob in range(c.OB):
                nc.vector.memset(cqTz[0][ob][D:P, :], 0.0)
                nc.vector.memset(cqTz[1][ob][0:D, :], 0.0)
            for vt in cvv:
                nc.gpsimd.memset(vt[:, :, :, D:D + 1], 1.0)
            ca_wk_t = load_w_in(p_kvx, 'ca_wk', c.CFB)
            ca_wv_t = load_w_in(p_kvx, 'ca_wv', c.CFB)
            p_w1 = tc.alloc_tile_pool(name="p_w1", bufs=1)
            pre0 = []
            for k in range(4):
                t = p_w1.tile([P, c.F], BF16, tag="pre0", bufs=4,
                              name=f"pre0_{k}")
                off = k * P * c.F
                nc.sync.dma_start(
                    t[:], g16('x_mine')[off:off + P * c.F].rearrange(
                        "(p f) -> p f", f=c.F))
                pre0.append(t)
            sa_wq_t = load_w_in(p_w1, 'sa_wq', c.FB)
            sa_wk_t = load_w_in(p_w1, 'sa_wk', c.FB)
            sa_wv_t = load_w_in(p_w1, 'sa_wv', c.FB)

            # =====================================================
            # Stats helpers (shared by the LN phases)
            # =====================================================
            def stat_rows_for_group(pre, ptr, pst, pps, cols, grows,
                                    strow_tag="strow", strow_bufs=1):
                """cols: list of [128,2] stat tiles -> broadcast mean_b/rstd_b
                [128, grows] tiles."""
                strow_ps = pps.tile([P, grows], F32, tag=strow_tag,
                                    bufs=strow_bufs, name=pre + "strow")
                for k, col in enumerate(cols):
                    nc.tensor.transpose(strow_ps[0:33, k * P:(k + 1) * P],
                                        col[:], ident[:])
                mean_row = ptr.tile([1, grows], BF16, tag="mrow", bufs=2,
                                    name=pre + "mrow")
                nc.vector.tensor_copy(mean_row[:], strow_ps[0:1, :])
                rstd_row = ptr.tile([1, grows], BF16, tag="rrow", bufs=2,
                                    name=pre + "rrow")
                nc.vector.tensor_copy(rstd_row[:], strow_ps[32:33, :])
                mean_b = ptr.tile([P, grows], BF16, tag="mb", bufs=2,
                                  name=pre + "mb")
                _pbcast(nc, mean_b[:], mean_row[:])
                rstd_b = ptr.tile([P, grows], BF16, tag="rb", bufs=2,
                                  name=pre + "rb")
                _pbcast(nc, rstd_b[:], rstd_row[:])
                return mean_b, rstd_b

            def norm_zt(pre, ptr, j, xsl, mean_b, rstd_b, grows, dst=None):
                """z^T[j] = (x^T[j] - mean_b) * rstd_b; into dst (fp8) or a
                fresh bf16 tile."""
                t1 = ptr.tile([P, grows], BF16, tag="t1", bufs=2,
                              name=f"{pre}t1_{j}")
                nc.vector.tensor_tensor(t1[:], xsl, mean_b[:],
                                        op=AX.subtract)
                if dst is not None:
                    z = ptr.tile([P, grows], BF16, tag="z", bufs=2,
                                 name=f"{pre}z_{j}")
                    nc.vector.tensor_tensor(z[:], t1[:], rstd_b[:],
                                            op=AX.mult)
                    nc.scalar.copy(dst, z[:])
                    return None
                z = ptr.tile([P, grows], BF16, tag="z", bufs=2,
                             name=f"{pre}z_{j}")
                nc.vector.tensor_tensor(z[:], t1[:], rstd_b[:], op=AX.mult)
                return z

            # =====================================================
            # Phase: LN(transposed) + q/k/v projections (generic)
            # =====================================================
            def ln_proj_phase(pre, fb_n, g_t, b_t, gq_t, bq_t, wk_t, wv_t,
                              wq_t, kT_l, v_l, qT_l, srcs, xt_name, mtot,
                              q_rows):
                fp_n = fb_n // 2
                with tc.tile_pool(name=pre + "tr", bufs=1) as ptr, \
                     tc.tile_pool(name=pre + "st", bufs=8) as pst, \
                     tc.tile_pool(name=pre + "ps", bufs=1, space="PSUM") as pps:
                    wkv = wk_t[:].rearrange("p (a o) -> p a o", a=fb_n)
                    wvv = wv_t[:].rearrange("p (a o) -> p a o", a=fb_n)
                    wqv = (wq_t[:].rearrange("p (a o) -> p a o", a=fb_n)
                           if wq_t is not None else None)
                    gi = 0  # global tile index
                    for (kind, src, ntiles, base) in srcs:
                        si = 0
                        while si < ntiles:
                            gs = min(4, ntiles - si)
                            grows = gs * P
                            goff = gi * P
                            # --- stats for the group's rows ---
                            cols = []
                            for k in range(gs):
                                if kind == 'dram':
                                    xt_t = ptr.tile([P, fb_n * P], BF16,
                                                    tag="xt", bufs=4,
                                                    name=pre + "xt")
                                    fd = fb_n * P
                                    off = (base + si + k) * P * fd
                                    nc.sync.dma_start(
                                        xt_t[:],
                                        g16(src)[off:off + P * fd].rearrange(
                                            "(p f) -> p f", f=fd))
                                    xt = xt_t[:]
                                else:
                                    xt = src[si + k][:]
                                col = ptr.tile([P, 33], F32, tag="stc",
                                               bufs=8, name=pre + "stc")
                                _stats_cols(nc, pst, xt, fb_n * P, eps_t, col)
                                cols.append(col)
                            mean_b, rstd_b = stat_rows_for_group(
                                pre, ptr, pst, pps, cols, grows)
                            # --- z^T + kv affine (+ q affine) per f-block,
                            #     into fp8 pair tiles, then projections ---
                            cn = [ptr.tile([P, 2, grows], FP8,
                                           tag=f"cn{jp}", bufs=2,
                                           name=f"{pre}cn{jp}")
                                  for jp in range(fp_n)]
                            qn = cn
                            if has_bias and goff < q_rows:
                                qn = [ptr.tile([P, 2, grows], FP8,
                                               tag=f"qn{jp}", bufs=2,
                                               name=f"{pre}qn{jp}")
                                      for jp in range(fp_n)]
                            xtj8 = ptr.tile([P, fb_n, grows], BF16,
                                            tag="xtj8", bufs=3,
                                            name=pre + "xtj8")
                            nc.sync.dma_start(
                                xtj8[:],
                                g16(xt_name).rearrange(
                                    "(a p m) -> p a m", p=P, m=mtot)[
                                    :, :, goff:goff + grows])
                            for j in range(fb_n):
                                if not has_bias:
                                    # gains are folded into the fp8 weights
                                    norm_zt(pre, ptr, j, xtj8[:, j, :],
                                            mean_b, rstd_b, grows,
                                            dst=cn[j // 2][:, j % 2, :])
                                    continue
                                z = norm_zt(pre, ptr, j, xtj8[:, j, :],
                                            mean_b, rstd_b, grows)
                                nc.scalar.activation(
                                    cn[j // 2][:, j % 2, :], z[:],
                                    AF.Identity, bias=b_t[:, j:j + 1],
                                    scale=g_t[:, j:j + 1])
                                if goff < q_rows:
                                    nc.scalar.activation(
                                        qn[j // 2][:, j % 2, :], z[:],
                                        AF.Identity, bias=bq_t[:, j:j + 1],
                                        scale=gq_t[:, j:j + 1])
                            # --- k^T projection (fp8 DoubleRow) ---
                            for ob in range(c.OB):
                                ktp = pps.tile([P, grows], F32, tag="ktp",
                                               bufs=2, name=pre + "ktp")
                                for jp in range(fp_n):
                                    nc.tensor.matmul(
                                        ktp[:],
                                        wkv[:, 2 * jp:2 * jp + 2,
                                            ob * P:(ob + 1) * P],
                                        cn[jp][:],
                                        start=(jp == 0),
                                        stop=(jp == fp_n - 1),
                                        perf_mode=DR)
                                nc.scalar.copy(
                                    kT_l[ob][:, goff:goff + grows], ktp[:])
                            # --- v projection (per m-tile, fp8 DoubleRow) ---
                            for k in range(gs):
                                vp = pps.tile([P, c.MID], F32, tag="vp",
                                              bufs=2, name=pre + "vp")
                                for jp in range(fp_n):
                                    nc.tensor.matmul(
                                        vp[:],
                                        cn[jp][:, :, k * P:(k + 1) * P],
                                        wvv[:, 2 * jp:2 * jp + 2, :],
                                        start=(jp == 0),
                                        stop=(jp == fp_n - 1),
                                        perf_mode=DR)
                                mi = gi + k
                                vt = v_l[mi // 2]
                                nc.scalar.activation(
                                    vt[:, :, mi % 2, 0:D],
                                    vp[:].rearrange("p (h x) -> p h x", x=D),
                                    AF.Copy, scale=1.0 / WS)
                            # --- q^T projection (fp8 DoubleRow) ---
                            if goff < q_rows:
                                for ob in range(c.OB):
                                    qtp = pps.tile([P, grows], F32, tag="qtp",
                                                   bufs=2, name=pre + "qtp")
                                    for jp in range(fp_n):
                                        nc.tensor.matmul(
                                            qtp[:],
                                            wqv[:, 2 * jp:2 * jp + 2,
                                                ob * P:(ob + 1) * P],
                                            qn[jp][:],
                                            start=(jp == 0),
                                            stop=(jp == fp_n - 1),
                                            perf_mode=DR)
                                    nc.scalar.copy(
                                        qT_l[0][ob][0:D, goff:goff + grows],
                                        qtp[0:D, :])
                                    nc.scalar.copy(
                                        qT_l[1][ob][D:P, goff:goff + grows],
                                        qtp[D:P, :])
                            si += gs
                            gi += gs

            # =====================================================
            # Phase: attention (generic); sink(tci, otp, psc) per t-chunk
            # =====================================================
            def attn_phase(pre, mt_n, kT_l, v_l, qT_l, sink,
                           after_chunk=None, chunks=None, psc_ext=None):
                mp_n = mt_n // 2
                with tc.tile_pool(name=pre + "at", bufs=1) as pat:
                    psc = psc_ext if psc_ext is not None else \
                        tc.alloc_tile_pool(name=pre + "sps", bufs=1,
                                           space="PSUM")
                    for tci in (chunks if chunks is not None
                                else range(c.NTC)):
                        toff = tci * c.TCHUNK
                        # fp8 attn-out pair tiles [P, 2, TCHUNK]
                        otp = [pat.tile([P, 2, c.TCHUNK], FP8, tag="ot",
                                        bufs=c.OB, name=pre + "ot")
                               for _ in range(c.OB // 2)]
                        # scores bf16 full-shape (zero-banded q); PV fp8
                        # DoubleRow over m-pairs with ones-column denominator
                        for h in range(c.H):
                            ob, par, hp = h // 2, h % 2, (h % 2) * D
                            pv = psc.tile([P, c.TCHUNK], F32, tag="pv",
                                          bufs=2, name=pre + "pv")
                            for pi in range(mp_n):
                                sps = psc.tile([P, 2 * c.TCHUNK], F32,
                                               tag="sps", bufs=2,
                                               name=pre + "sps")
                                for k in range(2):
                                    mi = 2 * pi + k
                                    nc.tensor.matmul(
                                        sps[:, k * c.TCHUNK:
                                            (k + 1) * c.TCHUNK],
                                        kT_l[ob][:, mi * P:(mi + 1) * P],
                                        qT_l[par][ob][:,
                                                      toff:toff + c.TCHUNK],
                                        start=True, stop=True)
                                et = pat.tile([P, 2 * c.TCHUNK], FP8,
                                              tag="et", bufs=6,
                                              name=pre + "et")
                                nc.scalar.activation(
                                    et[:], sps[:], AF.Exp,
                                    scale=ESCALE, bias=ebias_t[:])
                                nc.tensor.matmul(
                                    pv[:],
                                    v_l[pi][:, h, :, :],
                                    et[:].rearrange("p (a n) -> p a n", a=2),
                                    start=(pi == 0), stop=(pi == mp_n - 1),
                                    perf_mode=DR)
                            rr = pat.tile([1, c.TCHUNK], F32, tag="rr",
                                          bufs=3, name=pre + "rr")
                            nc.vector.tensor_copy(rr[:], pv[64:65, :])
                            rcp = pat.tile([1, c.TCHUNK], F32, tag="rcp",
                                           bufs=3, name=pre + "rcp")
                            nc.vector.reciprocal_approx_fast(
                                out=rcp[:], in_=rr[:])
                            rcb = pat.tile([D, c.TCHUNK], F32, tag="rcb",
                                           bufs=3, name=pre + "rcb")
                            _pbcast(nc, rcb[:], rcp[:])
                            nc.vector.scalar_tensor_tensor(
                                otp[ob // 2][hp:hp + D, ob % 2, :],
                                pv[0:D, :],
                                OTS, rcb[:], op0=AX.mult, op1=AX.mult)
                        sink(tci, otp, psc)
                        if after_chunk is not None:
                            after_chunk(tci, psc)
                    if psc_ext is None:
                        psc.release()

            def out_proj(pre, pop, otp, wov, tci, row_sink):
                for tb in range(TPC):
                    idx = tci * TPC + tb
                    for n2 in range(NC2):
                        opp = pop.tile([P, NCW], F32, tag="opp", bufs=2,
                                       name=pre + "opp")
                        for g in range(c.OB // 2):
                            nc.tensor.matmul(
                                opp[:],
                                otp[g][:, :, tb * P:(tb + 1) * P],
                                wov[:, 2 * g:2 * g + 2,
                                    n2 * NCW:(n2 + 1) * NCW],
                                start=(g == 0), stop=(g == c.OB // 2 - 1),
                                perf_mode=DR)
                        row_sink(idx, n2, opp)

            ln_proj_phase("c0", c.CFB, ca_gkv_t, ca_bkv_t, None, None,
                          ca_wk_t, ca_wv_t, None, ckT, cvv, None,
                          [('dram', 'ctx', c.CTB, 0)], 'ctxT', c.MC, 0)

            # ============ SELF-ATTENTION ============
            ln_proj_phase("s1", c.FB, sa_gkv_t, sa_bkv_t, sa_gq_t, sa_bq_t,
                          sa_wk_t, sa_wv_t, sa_wq_t, kT, vv, qTz,
                          [('sbuf', pre0, 4, 0),
                           ('dram', 'x_mine', c.TB - 4, 4),
                           ('dram', 'x_other', c.TB, 0)],
                          'xT', c.M, c.T)
            p_w1.release()

            # late-needed weights: issued after S1 so they don't queue ahead
            # of the S1 activation loads
            sa_wo_t = load_w_out(p_wl, 'sa_wo')
            ca_wq_t = load_w_in(p_wl, 'ca_wq', c.FB)
            ca_wo_t = load_w_out(p_wl, 'ca_wo')
            sa_wo_v = sa_wo_t[:].rearrange("p (a f) -> p a f", a=c.OB)
            ca_wo_v = ca_wo_t[:].rearrange("p (a f) -> p a f", a=c.OB)

            # x1 ([t,F] fp32) and x1^T ([F,t] bf16) live to the end
            p_x1 = tc.alloc_tile_pool(name="p_x1", bufs=1)
            x1 = [p_x1.tile([P, c.F], BF16, tag="x1", bufs=c.TB,
                            name=f"x1_{i}") for i in range(c.TB)]
            x1T = [p_x1.tile([P, c.T], BF16, tag="x1T", bufs=c.FB,
                             name=f"x1T_{j}") for j in range(c.FB)]
            p_sink = tc.alloc_tile_pool(name="p_sink", bufs=1)
            sa_bo_row = p_sink.tile([1, c.F], BF16, name="sa_bo_row")
            nc.sync.dma_start(sa_bo_row[:],
                              g16('sa_bo16').rearrange("(a f) -> a f", a=1))
            sa_bo_b = p_sink.tile([P, c.F], BF16, name="sa_bo_b")
            _pbcast(nc, sa_bo_b[:], sa_bo_row[:])
            ca_bo_row = p_x1.tile([1, c.F], BF16, name="ca_bo_row")
            nc.sync.dma_start(ca_bo_row[:],
                              g16('ca_bo16').rearrange("(a f) -> a f", a=1))
            ca_bo_b = p_x1.tile([P, c.F], BF16, name="ca_bo_b")
            _pbcast(nc, ca_bo_b[:], ca_bo_row[:])

            xb_cache = {}

            def self_row_sink(idx, n2, opp):
                # x1 = out_proj/256 + (x + sa_bo)
                if idx not in xb_cache:
                    xf = p_sink.tile([P, c.F], BF16, tag="xf", bufs=4,
                                     name="xf")
                    off = idx * P * c.F
                    nc.sync.dma_start(
                        xf[:],
                        g16('x_mine')[off:off + P * c.F].rearrange(
                            "(p f) -> p f", f=c.F))
                    xb = p_sink.tile([P, c.F], BF16, tag="xb", bufs=3,
                                     name="xb")
                    nc.vector.tensor_tensor(xb[:], xf[:], sa_bo_b[:],
                                            op=AX.add)
                    xb_cache[idx] = xb
                xb = xb_cache[idx]
                sl = slice(n2 * NCW, (n2 + 1) * NCW)
                nc.vector.scalar_tensor_tensor(
                    x1[idx][:, sl], opp[:], SINKS, xb[:, sl],
                    op0=AX.mult, op1=AX.add)

            def self_sink(tci, otp, psc):
                toff = tci * c.TCHUNK
                out_proj("s2", psc, otp, sa_wo_v, tci, self_row_sink)
                # transposed out-proj -> x1^T chunk (fp8 DoubleRow)
                for j in range(c.FB):
                    optp = psc.tile([P, c.TCHUNK], F32, tag="opp",
                                    bufs=2, name="optT")
                    for g in range(c.OB // 2):
                        nc.tensor.matmul(
                            optp[:],
                            sa_wo_v[:, 2 * g:2 * g + 2,
                                    j * P:(j + 1) * P],
                            otp[g][:],
                            start=(g == 0), stop=(g == c.OB // 2 - 1),
                            perf_mode=DR)
                    t2 = p_sink.tile([P, c.TCHUNK], F32, tag="t2", bufs=2,
                                     name="t2")
                    nc.vector.tensor_scalar(
                        t2[:], optp[:], SINKS, sa_bo_col[:, j:j + 1],
                        op0=AX.mult, op1=AX.add)
                    xTs = g16('xT').rearrange("(f m) -> f m", m=c.M)[
                        j * P:(j + 1) * P, toff:toff + c.TCHUNK]
                    xTj = p_sink.tile([P, c.TCHUNK], BF16, tag="xTj", bufs=4,
                                      name="xTj")
                    nc.sync.dma_start(xTj[:], xTs)
                    nc.vector.tensor_tensor(
                        x1T[j][:, toff:toff + c.TCHUNK], t2[:], xTj[:],
                        op=AX.add)

            # x1 LN + cross-q projection, one group per self chunk,
            # interleaved into S2 via after_chunk (PSUM tags shared with
            # the attention pool: qtp->"pv", strow->"opp")
            c1tr = tc.alloc_tile_pool(name="c1tr", bufs=1)
            c1st = tc.alloc_tile_pool(name="c1st", bufs=8)
            cwqv = ca_wq_t[:].rearrange("p (a o) -> p a o", a=c.FB)

            def c1_group(tci, psc):
                g0 = tci * TPC
                gs = min(TPC, c.TB - g0)
                grows = gs * P
                goff = g0 * P
                cols = []
                for k in range(gs):
                    col = c1tr.tile([P, 33], F32, tag="stc", bufs=8,
                                    name="c1stc")
                    _stats_cols(nc, c1st, x1[g0 + k][:], c.F, eps_t, col)
                    cols.append(col)
                # pre-bias the residual once stats are taken: x1 += ca_bo
                for k in range(gs):
                    nc.vector.tensor_tensor(
                        x1[g0 + k][:], x1[g0 + k][:], ca_bo_b[:],
                        op=AX.add)
                mean_b, rstd_b = stat_rows_for_group(
                    "c1", c1tr, c1st, psc, cols, grows, strow_tag="opp",
                    strow_bufs=2)
                qn = [c1tr.tile([P, 2, grows], FP8, tag=f"qn{jp}", bufs=1,
                                name=f"c1qn{jp}") for jp in range(c.FB // 2)]
                for j in range(c.FB):
                    if not has_bias:
                        norm_zt("c1", c1tr, j,
                                x1T[j][:, goff:goff + grows],
                                mean_b, rstd_b, grows,
                                dst=qn[j // 2][:, j % 2, :])
                        continue
                    z = norm_zt("c1", c1tr, j,
                                x1T[j][:, goff:goff + grows],
                                mean_b, rstd_b, grows)
                    nc.scalar.activation(
                        qn[j // 2][:, j % 2, :], z[:], AF.Identity,
                        bias=ca_bq_t[:, j:j + 1],
                        scale=ca_gq_t[:, j:j + 1])
                for ob in range(c.OB):
                    qtp = psc.tile([P, grows], F32, tag="pv", bufs=2,
                                   name="c1qtp")
                    for jp in range(c.FB // 2):
                        nc.tensor.matmul(
                            qtp[:],
                            cwqv[:, 2 * jp:2 * jp + 2,
                                 ob * P:(ob + 1) * P],
                            qn[jp][:],
                            start=(jp == 0), stop=(jp == c.FB // 2 - 1),
                            perf_mode=DR)
                    nc.scalar.copy(
                        cqTz[0][ob][0:D, goff:goff + grows], qtp[0:D, :])
                    nc.scalar.copy(
                        cqTz[1][ob][D:P, goff:goff + grows], qtp[D:P, :])

            attn_phase("s2", c.MT, kT, vv, qTz, self_sink)

            # ============ CROSS-ATTENTION ============
            def cross_row_sink(idx, n2, opp):
                # out = out_proj/256 + (x1 + ca_bo)  [bias pre-folded]
                sl = slice(n2 * NCW, (n2 + 1) * NCW)
                o2 = p_x1.tile([P, NCW], F32, tag="o2", bufs=3, name="o2")
                nc.vector.scalar_tensor_tensor(
                    o2[:], opp[:], SINKS, x1[idx][:, sl],
                    op0=AX.mult, op1=AX.add)
                nc.sync.dma_start(
                    out_d.ap().rearrange(
                        "(tb p) f -> tb p f", p=P)[idx][:, sl],
                    o2[:])

            def cross_sink(tci, otp, psc):
                out_proj("c2", psc, otp, ca_wo_v, tci, cross_row_sink)

            cps = tc.alloc_tile_pool(name="cps", bufs=1, space="PSUM")
            c1_group(0, cps)
            c1_group(1, cps)
            attn_phase("c2", c.CTB, ckT, cvv, cqTz, cross_sink,
                       psc_ext=cps)
            cps.release()
            c1st.release()
            c1tr.release()
            p_sink.release()

            p_x1.release()
            p_kvx.release()
            p_wl.release()

    return nc


# ---------------------------------------------------------------------------
# host-side: shard, run, gather
# ---------------------------------------------------------------------------

def ln_has_bias(params):
    return any(np.any(np.asarray(params[k], np.float32))
               for k in ('sa_nb', 'sa_ncb', 'ca_nb', 'ca_ncb'))


def raw_core_inputs(cfg, x, context, params, n_cores=8):
    bf = ml_dtypes.bfloat16
    f8 = ml_dtypes.float8_e4m3
    c = cfg

    def t_ln(v, fb):
        return np.ascontiguousarray(
            np.asarray(v, np.float32).reshape(fb, P).T)

    def q8(w, s, g=None):
        w = np.asarray(w, np.float32)
        if g is not None:  # fold LN gain into the weight rows
            w = w * np.asarray(g, np.float32)[:, None]
        return np.clip(w * s, -240, 240).astype(f8)

    hb = ln_has_bias(params)
    g_sq = None if hb else params['sa_ng']
    g_sk = None if hb else params['sa_ncg']
    g_cq = None if hb else params['ca_ng']
    g_ck = None if hb else params['ca_ncg']
    shared = {
        'sa_wq': q8(params['sa_wq'], WS, g_sq),
        'sa_wk': q8(params['sa_wkv'][:, :c.MID], WS, g_sk),
        'sa_wv': q8(params['sa_wkv'][:, c.MID:], WS, g_sk),
        'sa_wo': q8(params['sa_wo'], WOS),
        'ca_wq': q8(params['ca_wq'], WS, g_cq),
        'ca_wk': q8(params['ca_wkv'][:, :c.MID], WS, g_ck),
        'ca_wv': q8(params['ca_wkv'][:, c.MID:], WS, g_ck),
        'ca_wo': q8(params['ca_wo'], WOS),
        'sa_gq': t_ln(params['sa_ng'], c.FB),
        'sa_bq': t_ln(params['sa_nb'], c.FB),
        'sa_gkv': t_ln(params['sa_ncg'], c.FB),
        'sa_bkv': t_ln(params['sa_ncb'], c.FB),
        'ca_gq': t_ln(params['ca_ng'], c.FB),
        'ca_bq': t_ln(params['ca_nb'], c.FB),
        'ca_gkv': t_ln(params['ca_ncg'], c.CFB),
        'ca_bkv': t_ln(params['ca_ncb'], c.CFB),
        'sa_bo': np.asarray(params['sa_bo'], np.float32).reshape(1, c.F),
        'ca_bo': np.asarray(params['ca_bo'], np.float32).reshape(1, c.F),
        'sa_bo16': np.asarray(params['sa_bo'], np.float32).astype(
            bf).reshape(1, c.F),
        'ca_bo16': np.asarray(params['ca_bo'], np.float32).astype(
            bf).reshape(1, c.F),
        'sa_bo_col': t_ln(params['sa_bo'], c.FB),
        'ca_bo_col': t_ln(params['ca_bo'], c.FB),
    }
    n_batch = x.shape[0]
    in_maps = []
    for core in range(n_cores):
        b, th = core // 2, core % 2
        b = min(b, n_batch - 1)
        m = dict(shared)
        xm = np.ascontiguousarray(
            x[b, th * c.T:(th + 1) * c.T]).astype(bf)
        xo = np.ascontiguousarray(
            x[b, (1 - th) * c.T:(2 - th) * c.T]).astype(bf)
        m['x_mine'] = xm
        m['x_other'] = xo
        m['ctx'] = np.ascontiguousarray(context[b]).astype(bf)
        m['xT'] = np.ascontiguousarray(
            np.concatenate([xm, xo], 0).T)
        m['ctxT'] = np.ascontiguousarray(m['ctx'].T)
        in_maps.append(m)
    return in_maps


def pack_core_inputs(cfg, raws):
    L32, N32 = layout32(cfg)
    L16, N16 = layout16(cfg)
    L8, N8 = layout8(cfg)
    packed = []
    for im in raws:
        b32 = np.zeros(N32, np.float32)
        for name, (off, size) in L32.items():
            b32[off:off + size] = np.asarray(im[name], np.float32).ravel()
        b16 = np.empty(N16, ml_dtypes.bfloat16)
        for name, (off, size) in L16.items():
            b16[off:off + size] = np.asarray(im[name]).ravel()
        b8 = np.empty(N8, ml_dtypes.float8_e4m3)
        for name, (off, size) in L8.items():
            b8[off:off + size] = np.asarray(im[name]).ravel()
        packed.append({'blob32': b32, 'blob16': b16, 'blob8': b8})
    return packed


def prep_core_inputs(cfg, x, context, params, n_cores=8):
    return pack_core_inputs(
        cfg, raw_core_inputs(cfg, x, context, params, n_cores))


def build_dummy(nc, cfg):
    c = cfg
    L32, N32 = layout32(c)
    L16, N16 = layout16(c)
    L8, N8 = layout8(c)
    nc.dram_tensor("blob32", [N32], F32, kind="ExternalInput")
    nc.dram_tensor("blob16", [N16], BF16, kind="ExternalInput")
    nc.dram_tensor("blob8", [N8], FP8, kind="ExternalInput")
    out_d = nc.dram_tensor("out", [c.T, c.F], F32, kind="ExternalOutput")
    with tile.TileContext(nc) as tc:
        with tc.tile_pool(name="pd", bufs=1) as pd:
            dz = pd.tile([P, c.F], F32, name="dz")
            nc.vector.memset(dz[:], 0.0)
            for i in range(c.TB):
                nc.sync.dma_start(
                    out_d.ap().rearrange("(tb p) f -> tb p f", p=P)[i], dz[:])
    return nc


_CACHED = {}


def get_nc(cfg, num_devices=8, has_bias=False):
    key = (cfg.F, cfg.CF, cfg.T, cfg.MC, cfg.H, num_devices, has_bias)
    if key not in _CACHED:
        nc = bacc.Bacc("TRN2", target_bir_lowering=False, debug=False,
                       num_devices=num_devices)
        build(nc, cfg, has_bias=has_bias)
        nc.compile()
        _CACHED[key] = nc
    return _CACHED[key]


def get_dummy_nc(cfg, num_devices=8):
    key = ('dummy', cfg.F, cfg.T, num_devices)
    if key not in _CACHED:
        nc = bacc.Bacc("TRN2", target_bir_lowering=False, debug=False,
                       num_devices=num_devices)
        build_dummy(nc, cfg)
        nc.compile()
        _CACHED[key] = nc
    return _CACHED[key]


def kernel(x, context,
           sa_ng, sa_nb, sa_ncg, sa_ncb, sa_wq, sa_wkv, sa_wo, sa_bo,
           ca_ng, ca_nb, ca_ncg, ca_ncb, ca_wq, ca_wkv, ca_wo, ca_bo):
    from concourse import bass_utils
    cfg = Cfg()
    params = dict(sa_ng=sa_ng, sa_nb=sa_nb, sa_ncg=sa_ncg, sa_ncb=sa_ncb,
                  sa_wq=sa_wq, sa_wkv=sa_wkv, sa_wo=sa_wo, sa_bo=sa_bo,
                  ca_ng=ca_ng, ca_nb=ca_nb, ca_ncg=ca_ncg, ca_ncb=ca_ncb,
                  ca_wq=ca_wq, ca_wkv=ca_wkv, ca_wo=ca_wo, ca_bo=ca_bo)
    x = np.asarray(x)
    context = np.asarray(context)
    params = {k: np.asarray(v) for k, v in params.items()}
    in_maps = prep_core_inputs(cfg, x, context, params)
    nc = get_nc(cfg, has_bias=ln_has_bias(params))
    res = bass_utils.run_bass_kernel_spmd(nc, in_maps, core_ids=list(range(8)))
    out = np.empty((4, 2048, 1024), np.float32)
    for core in range(8):
        b, th = core // 2, core % 2
        out[b, th * cfg.T:(th + 1) * cfg.T] = res.results[core]['out']
    return out
